# revision 19
# baseline (speedup 1.0000x reference)
"""AdaProj kernel v3 for 8 TRN2 NeuronCores.

Math per class c, sample b (C-sharded 125 classes/core, x replicated):
  L_s[c,b] = W[c,s,:] . x[b,:]              (PE, 4 k-chunks)
  nsq[c,s] = ||W[c,s]||^2, Gq[c,j] = W_s.W_s'   (products + free-1 matmuls)
  alpha_s = 1/nsq_s ; beta_j = 2*alpha_s*alpha_s'*Gq_j
  num = sum_s alpha_s L_s^2 ; den = num + sum_j beta_j L_s L_s'
  out[c,b] = rnx_b * num / sqrt(den),  rnx = 1/||x_b||

v3 vs v2:
  - alpha/beta formulation: no sqrt (rnw) on the critical path; L copied
    PSUM->SBUF fp16 unscaled, then 10 fused scalar_tensor_tensor ops
    (in0*coef)*in1 produce all scaled quadratic terms on DVE.
  - num/den accumulated on PE via identity matmuls into one PSUM bank.
  - Output via kv_writeback: descriptors prepared (gen_mode=1) right after
    the final ot write, fired by trigger_dma -> the transfer skips the HWDGE
    descriptor stage (625ns) and DGE ramp (650ns) on the critical tail.
  - Input: chunks 0,1,2 via SP HWDGE, chunk 3 via Pool SWDGE (inline), so
    arrival order is 0,3,1,2; all per-chunk work is emitted in that order.
"""

import numpy as np

import concourse.bacc as bacc
import concourse.bass as bass
import concourse.mybir as mybir
import concourse.tile as tile
from concourse.bass_utils import run_bass_kernel_spmd

B, C, S, D = 256, 1000, 4, 512
NCORES = 8
CS = C // NCORES          # 125 classes per core
R = CS * S                # 500 W rows per core
KP = D // 128             # 4 contraction chunks
XWC = B + R               # 756 packed cols: 256 x | 500 W

F32 = mybir.dt.float32
FP16 = mybir.dt.float16
I32 = mybir.dt.int32
AF = mybir.ActivationFunctionType
OP = mybir.AluOpType

N_WARM = 18  # dummy PE matmuls holding the p-state ramp until data lands

KORD = [0, 3, 1, 2]       # chunk arrival order (SP: 0,1,2; Pool: 3)
KLAST = KORD[-1]

# quadratic cross-term pairs (s, s') and their beta/gq column j
PAIRS = [(0, 1), (1, 2), (2, 3), (0, 2), (1, 3), (0, 3)]

_CACHED = {}


def _emit_body(nc, pool, psum):
    xw_d = nc.dram_tensor("xw", [D, XWC], FP16, kind="ExternalInput")
    out_d = nc.dram_tensor("out", [1, 128, 1, B], FP16, kind="ExternalOutput")

    def st(shape, dtype, name, space_pool=None):
        sp = space_pool if space_pool is not None else pool
        return sp.tile(shape, dtype, tag=name, name=name)

    def mm(out, lhsT, rhs, start, stop):
        return nc.tensor.matmul(out, lhsT, rhs, start=start, stop=stop,
                                skip_group_check=True)

    def dep(a, b, reason):
        bass._add_dep_helper(a.ins, b.ins, sync=False, reason=reason)

    def mm_chain(prev, out, lhsT, rhs, start, stop):
        i = mm(out, lhsT, rhs, start, stop)
        if prev is not None:
            dep(i, prev, "psum accumulation group order")
        return i

    # ---------------- PSUM banks (2KB each, one start-opener per bank) ----
    Lq01 = st([CS, 2, B], F32, "Lq01", psum)   # bank: L_0 | L_1
    Lq23 = st([CS, 2, B], F32, "Lq23", psum)   # bank: L_2 | L_3
    bankC = st([128, 512], F32, "bankC", psum)  # nsq | gq
    nsq = bankC[0:CS, 0:S]
    gq = bankC[0:CS, S:S + 6]
    bankX = st([128, 512], F32, "bankX", psum)  # nx row | rnx_bc
    nx = bankX[0:1, 0:B]
    rnx_bc = bankX[0:CS, B:2 * B]
    numden = st([CS, 2, B], F32, "numden", psum)  # num | den
    numb = numden[:, 0, :]
    denb = numden[:, 1, :]

    def Lsl(s):
        return (Lq01 if s < 2 else Lq23)[:, s % 2, :]

    # ---------------- tiny init + act table warm ----------------
    warm = st([1, 1], F32, "warm")
    nc.vector.memset(warm[:], 1.0)
    warm3 = st([1, 1], F32, "warm3")
    nc.scalar.activation(warm3[:], warm[:], AF.Abs_reciprocal_sqrt)

    ones_w = st([128, 1], FP16, "ones_w")
    nc.vector.memset(ones_w[:], 1.0)
    ones_row = st([1, 128], FP16, "ones_row")
    nc.vector.memset(ones_row[:], 1.0)
    dum = st([128, 128], FP16, "dum")
    nc.vector.memset(dum[:], 0.03)
    eye = st([CS, CS], FP16, "eye")
    nc.vector.memset(eye[:], 1.0)
    ctx0 = st([128, 1], I32, "ctx0")
    nc.vector.memset(ctx0[:], 0)
    ot = st([128, 1, 1, B], FP16, "ot")
    nc.vector.memset(ot[:], 0.0)

    # ---------------- PE warmup (bank-D openers, reset by num later) ------
    for _ in range(N_WARM):
        mm(numden[0:1, 0, 0:128], ones_w[:], dum[:], start=True, stop=True)

    # ---------------- input DMAs ----------------
    xw = st([128, KP, XWC], FP16, "xw")
    nc.gpsimd.dma_start(xw[:, 3, :], xw_d[3 * 128:4 * 128, :])
    for k in range(KP - 1):
        nc.sync.dma_start(xw[:, k, :], xw_d[k * 128:(k + 1) * 128, :])

    # eye diagonal (Pool, after the dma desc-gen)
    nc.gpsimd.affine_select(
        eye[:], eye[:], pattern=[[-1, CS]], compare_op=OP.is_equal,
        fill=0.0, base=0, channel_multiplier=1,
    )

    def wsl(k, lo, hi):
        return xw[:, k, B + lo:B + hi]

    # ---------------- per-k products (arrival order; last chunk on DVE) ---
    prodD = st([128, KP, R], FP16, "prodD")
    prodA = st([128, KP, 3 * CS], FP16, "prodA")   # (0,1),(1,2),(2,3)
    prodB = st([128, KP, 2 * CS], FP16, "prodB")   # (0,2),(1,3)
    prodC = st([128, KP, CS], FP16, "prodC")       # (0,3)
    xsq = st([128, KP, B], FP16, "xsq")
    for k in KORD:
        if k != KLAST:
            nc.scalar.square(prodD[:, k, :], wsl(k, 0, R))           # Act
            nc.vector.tensor_tensor(prodA[:, k, :], wsl(k, 0, 3 * CS),
                                    wsl(k, CS, R), OP.mult)          # DVE
            nc.vector.tensor_tensor(prodB[:, k, :], wsl(k, 0, 2 * CS),
                                    wsl(k, 2 * CS, R), OP.mult)      # DVE
        else:
            nc.vector.tensor_tensor(prodD[:, k, :], wsl(k, 0, R),
                                    wsl(k, 0, R), OP.mult)           # DVE
            nc.vector.tensor_tensor(prodA[:, k, :], wsl(k, 0, 3 * CS),
                                    wsl(k, CS, R), OP.mult)
            nc.vector.tensor_tensor(prodB[:, k, :], wsl(k, 0, 2 * CS),
                                    wsl(k, 2 * CS, R), OP.mult)
        nc.gpsimd.tensor_tensor(prodC[:, k, :], wsl(k, 0, CS),
                                wsl(k, 3 * CS, R), OP.mult)          # Pool
        nc.gpsimd.tensor_tensor(xsq[:, k, :], xw[:, k, 0:B],
                                xw[:, k, 0:B], OP.mult)              # Pool

    # ---------------- PE: L matmuls + reductions ----------------
    Lmm = [None] * S
    Nmm = [None] * S
    Gmm = [None] * 6
    nxmm = None
    copener = None  # bank-C opener = Nmm[0] first k
    xopener = None  # bank-X opener = nxmm first k

    def emit_reductions(ki, k):
        nonlocal nxmm, copener, xopener
        first = ki == 0
        for s in range(S):
            Nmm[s] = mm_chain(Nmm[s], bankC[0:CS, s:s + 1],
                              prodD[:, k, s * CS:(s + 1) * CS], ones_w[:],
                              start=(first and s == 0), stop=(ki == KP - 1))
            if copener is None:
                copener = Nmm[0]
            elif first and s > 0:
                dep(Nmm[s], copener, "bank opener first")
        for j in range(3):
            Gmm[j] = mm_chain(Gmm[j], bankC[0:CS, S + j:S + j + 1],
                              prodA[:, k, j * CS:(j + 1) * CS], ones_w[:],
                              start=False, stop=(ki == KP - 1))
            if first:
                dep(Gmm[j], copener, "bank opener first")
        for j in range(2):
            Gmm[3 + j] = mm_chain(Gmm[3 + j], bankC[0:CS, S + 3 + j:S + 4 + j],
                                  prodB[:, k, j * CS:(j + 1) * CS], ones_w[:],
                                  start=False, stop=(ki == KP - 1))
            if first:
                dep(Gmm[3 + j], copener, "bank opener first")
        Gmm[5] = mm_chain(Gmm[5], bankC[0:CS, S + 5:S + 6],
                          prodC[:, k, :], ones_w[:], start=False,
                          stop=(ki == KP - 1))
        if first:
            dep(Gmm[5], copener, "bank opener first")
        nxmm = mm_chain(nxmm, nx, ones_w[:], xsq[:, k, :],
                        start=first, stop=(ki == KP - 1))
        if first:
            xopener = nxmm

    for ki, k in enumerate(KORD[:-1]):
        for s in range(S):
            # start=True zeroes the whole 2KB bank: only s=0/s=2 open their
            # banks; s=1/s=3 accumulate into the opener-zeroed region.
            Lmm[s] = mm_chain(Lmm[s], Lsl(s), wsl(k, s * CS, (s + 1) * CS),
                              xw[:, k, 0:B],
                              start=(ki == 0 and s % 2 == 0), stop=False)
            if ki == 0 and s == 1:
                dep(Lmm[1], Lmm[0], "bank01 opener first")
            if ki == 0 and s == 3:
                dep(Lmm[3], Lmm[2], "bank23 opener first")
        emit_reductions(ki, k)

    # last chunk: L s0,s1 first, then nsq closers (prodD gated), then L
    # s2,s3, then remaining closers.
    k = KLAST
    for s in (0, 1):
        Lmm[s] = mm_chain(Lmm[s], Lsl(s), wsl(k, s * CS, (s + 1) * CS),
                          xw[:, k, 0:B], start=False, stop=True)
    for s in range(S):
        Nmm[s] = mm_chain(Nmm[s], bankC[0:CS, s:s + 1],
                          prodD[:, k, s * CS:(s + 1) * CS], ones_w[:],
                          start=False, stop=True)
    for s in (2, 3):
        Lmm[s] = mm_chain(Lmm[s], Lsl(s), wsl(k, s * CS, (s + 1) * CS),
                          xw[:, k, 0:B], start=False, stop=True)
    for j in range(3):
        Gmm[j] = mm_chain(Gmm[j], bankC[0:CS, S + j:S + j + 1],
                          prodA[:, k, j * CS:(j + 1) * CS], ones_w[:],
                          start=False, stop=True)
    for j in range(2):
        Gmm[3 + j] = mm_chain(Gmm[3 + j], bankC[0:CS, S + 3 + j:S + 4 + j],
                              prodB[:, k, j * CS:(j + 1) * CS], ones_w[:],
                              start=False, stop=True)
    Gmm[5] = mm_chain(Gmm[5], bankC[0:CS, S + 5:S + 6], prodC[:, k, :],
                      ones_w[:], start=False, stop=True)
    nxmm = mm_chain(nxmm, nx, ones_w[:], xsq[:, k, :], start=False, stop=True)

    # ---------------- coefficients (DVE smalls) ----------------
    alpha = st([CS, S], F32, "alpha")
    nc.vector.reciprocal(alpha[:], nsq)
    t6 = st([CS, 6], F32, "t6")
    nc.vector.tensor_tensor(t6[:, 0:3], alpha[:, 0:3], alpha[:, 1:4], OP.mult)
    nc.vector.tensor_tensor(t6[:, 3:5], alpha[:, 0:2], alpha[:, 2:4], OP.mult)
    nc.vector.tensor_tensor(t6[:, 5:6], alpha[:, 0:1], alpha[:, 3:4], OP.mult)
    beta = st([CS, 6], F32, "beta")
    nc.vector.scalar_tensor_tensor(
        out=beta[:], in0=gq, scalar=2.0, in1=t6[:], op0=OP.mult, op1=OP.mult)

    # ---------------- L PSUM -> SBUF fp16 (unscaled halves) ----------------
    mh = st([CS, S, B], FP16, "mh")
    nc.vector.tensor_scalar_add(mh[:, 0:2, :], Lq01[:, :, :], 0.0)   # DVE
    nc.scalar.copy(mh[:, 2:4, :], Lq23[:, :, :])                     # Act

    def ms(s):
        return mh[:, s, :]

    # ---------------- rnx path (Act + PE, off critical) ----------------
    rnx_row = st([1, B], FP16, "rnx_row")
    nc.scalar.activation(rnx_row[:], nx, AF.Abs_reciprocal_sqrt)

    # ---------------- fused scaled quadratic terms (DVE STTs) ----------
    # sp slices: 0..3 = P'_s (alpha_s * L_s^2), 4..9 = C'_j (beta_j L_s L_s')
    sp = st([CS, 10, B], FP16, "sp")
    dve_terms = [("P", 0), ("P", 1), ("C", 0), ("P", 2), ("P", 3),
                 ("C", 1), ("C", 2), ("C", 5), ("C", 3), ("C", 4)]

    for kind, j in dve_terms:
        if kind == "P":
            out_, in0, sc, in1 = sp[:, j, :], ms(j), alpha[:, j:j + 1], ms(j)
        else:
            s, s2 = PAIRS[j]
            out_, in0, sc, in1 = (sp[:, 4 + j, :], ms(s), beta[:, j:j + 1],
                                  ms(s2))
        nc.vector.scalar_tensor_tensor(out=out_, in0=in0, scalar=sc, in1=in1,
                                       op0=OP.mult, op1=OP.mult)

    # ---------------- num/den accumulation on PE (identity matmuls) -----
    nmm = mm(numb, eye[:], sp[:, 0, :], start=True, stop=False)
    dopener = nmm
    dmm = mm(denb, eye[:], sp[:, 0, :], start=False, stop=False)
    dep(dmm, dopener, "bank opener first")
    order = [("n", 1), ("d", 1), ("d", 4 + 0), ("n", 2), ("d", 2),
             ("n", 3), ("d", 3), ("d", 4 + 1), ("d", 4 + 2), ("d", 4 + 5),
             ("d", 4 + 3), ("d", 4 + 4)]
    rbc = None
    for which, idx in order:
        if which == "n":
            nmm = mm_chain(nmm, numb, eye[:], sp[:, idx, :],
                           start=False, stop=(idx == 3))
        else:
            dmm = mm_chain(dmm, denb, eye[:], sp[:, idx, :],
                           start=False, stop=(idx == 4 + 4))
        if which == "n" and idx == 3 and rbc is None:
            rbc = mm(rnx_bc, ones_row[:, 0:CS], rnx_row[:],
                     start=False, stop=True)
            dep(rbc, xopener, "bank opener first")

    # ---------------- tail ----------------
    rnx_sb = st([CS, B], FP16, "rnx_sb")
    nc.scalar.copy(rnx_sb[:], rnx_bc)
    u = st([CS, B], FP16, "u")
    nc.vector.tensor_tensor(u[:], numb, rnx_sb[:], OP.mult)
    srd = st([CS, B], FP16, "srd")
    nc.scalar.activation(srd[:], denb, AF.Abs_reciprocal_sqrt)
    nc.vector.tensor_tensor(ot[0:CS, 0, 0, :], u[:], srd[:], OP.mult)

    # output: SWDGE descriptors prepared AFTER the ot write (BIRSim reads
    # the source at prep position), fired by trigger_dma. The trigger-fired
    # transfer skips HWDGE+DGE-ramp latency on the critical tail.
    sem_ot = nc.alloc_semaphore("dma_ot")
    nc.gpsimd.kv_writeback(
        out_d[:, :, :, :], ot[:, :, :, :], ctx0[:],
        prepare_only=True, sem=sem_ot,
    )
    nc.gpsimd.trigger_dma(count=None)


def _build_nc():
    nc = bacc.Bacc(
        "TRN2",
        target_bir_lowering=False,
        debug=False,
        enable_asserts=False,
        num_devices=NCORES,
    )
    with tile.TileContext(nc) as tc:
        with (
            tc.tile_pool(name="psum", bufs=1, space="PSUM") as psum,
            tc.tile_pool(name="main", bufs=1) as pool,
        ):
            _emit_body(nc, pool, psum)
    nc.compile()
    _fix_swdge_waits(nc)
    return nc


def _fix_swdge_waits(nc):
    """Point consumer waits at the sems the SWDGE descriptors actually fire.

    Tile routes data deps on prepared-SWDGE outputs through per-lane DMASW<i>
    semaphores, but the hardware descriptor encodes exactly one sem — the
    user sem passed via ``sem=`` (on_update[0], +16 at transfer end). Tile
    never attaches the DMASW increment for gen_mode==1 preps, leaving DMASW
    waits unsatisfiable. Rewrite each unsatisfiable ``DMASW<i> >= 16`` wait
    to the user sem of the i-th prep (lanes assigned round-robin in prep
    program order).
    """
    import re

    fn = nc.m.functions[0]
    lane_sem = {}
    updated = set()
    n_dma = 0
    for blk in fn.blocks:
        for ins in blk.instructions:
            if ins.sync_info is None:
                continue
            for u in ins.sync_info.on_update:
                if u.ant_name:
                    updated.add(u.ant_name)
            if ins.engine == mybir.EngineType.Pool and (
                    type(ins).__name__ in ("InstDMACopy", "InstDMAGatherAnt",
                                           "InstKVWritebackAnt",
                                           "InstDMAScatterAddAnt",
                                           "InstPagedWritebackAnt")):
                if getattr(ins, "gen_mode", 0) == 1:
                    lane_sem[n_dma] = ins.sync_info.on_update[0]
                n_dma += 1
    for blk in fn.blocks:
        for ins in blk.instructions:
            if ins.sync_info is None:
                continue
            for w in ins.sync_info.on_wait:
                m = re.match(r"DMASW(\d+)_", w.ant_name or "")
                if not m or w.ant_name in updated:
                    continue
                lane = int(m.group(1))
                assert w.wait_value == 16, (ins.name, w.ant_name, w.wait_value)
                assert lane in lane_sem, (ins.name, w.ant_name, lane_sem)
                u = lane_sem[lane]
                w.id = u.id
                w.ant_name = u.ant_name


def _get_nc():
    if "nc" not in _CACHED:
        _CACHED["nc"] = _build_nc()
    return _CACHED["nc"]


def _make_in_maps(x, W):
    x = np.ascontiguousarray(np.asarray(x, dtype=np.float32))
    W = np.ascontiguousarray(np.asarray(W, dtype=np.float32))
    xT = x.T.astype(np.float16)  # [D, B]
    in_maps = []
    for i in range(NCORES):
        Ws = W[i * CS:(i + 1) * CS].astype(np.float16)      # [CS, S, D]
        wT = Ws.transpose(2, 1, 0).reshape(D, R)            # [D, s*CS+c]
        xw = np.ascontiguousarray(np.concatenate([xT, wT], axis=1))
        in_maps.append({"xw": xw})
    return in_maps


def run(x, W, trace=False):
    nc = _get_nc()
    in_maps = _make_in_maps(x, W)
    res = run_bass_kernel_spmd(
        nc, in_maps, core_ids=list(range(NCORES)), trace=trace
    )
    shards = []
    for i in range(NCORES):
        o = np.asarray(res.results[i]["out"]).reshape(128, B)[0:CS, :]
        shards.append(o.astype(np.float32))
    out = np.concatenate([s.T for s in shards], axis=1)  # [B, C]
    return np.ascontiguousarray(out.astype(np.float32)), res


def kernel(x, W):
    out, _ = run(x, W, trace=False)
    return out


# revision 23
# speedup vs baseline: 1.0513x; 1.0513x over previous
"""AdaProj kernel v4 for 8 TRN2 NeuronCores.

Math per class c, sample b (C-sharded 125 classes/core, x replicated):
  L_s[c,b] = W[c,s,:] . x[b,:]              (PE, 4 k-chunks)
  nsq[c,s] = ||W[c,s]||^2, Gq[c,j] = W_s.W_s'   (products + free-1 matmuls)
  alpha_s = 1/nsq_s ; rnw_s = sqrt(alpha_s) ; beta_j = 2*alpha_s*alpha_s'*Gq_j
  num = sum_s alpha_s L_s^2 ; den = num + sum_j beta_j L_s L_s'
  out[c,b] = rnx_b * num / sqrt(den),  rnx = 1/||x_b||

v4 scheduling structure:
  - Separate PSUM tiles per 2KB bank (start_tensor_calc zeroes a whole
    bank) and fine-grained SBUF tiles so Tile deps don't over-serialize.
  - Scaled quadratic terms: P'_0/P'_1 = alpha*(mh01)^2 via DVE square +
    scale; P'_2/P'_3 on Act as Square(rnw_s * L_s) straight from PSUM;
    cross terms as packed DVE pair-products + per-term 4x tensor_scalar.
  - num/den accumulated on PE via identity matmuls (one shared bank).
  - Output via kv_writeback prepared after the ot write + trigger_dma.
"""

import numpy as np

import concourse.bacc as bacc
import concourse.bass as bass
import concourse.mybir as mybir
import concourse.tile as tile
from concourse.bass_utils import run_bass_kernel_spmd

B, C, S, D = 256, 1000, 4, 512
NCORES = 8
CS = C // NCORES          # 125 classes per core
R = CS * S                # 500 W rows per core
KP = D // 128             # 4 contraction chunks
XWC = B + R               # 756 packed cols: 256 x | 500 W

F32 = mybir.dt.float32
FP16 = mybir.dt.float16
I32 = mybir.dt.int32
AF = mybir.ActivationFunctionType
OP = mybir.AluOpType

N_WARM = 18  # dummy PE matmuls holding the p-state ramp until data lands

KORD = [0, 3, 1, 2]       # chunk arrival order (SP: 0,1,2; Pool: 3)
KLAST = KORD[-1]

# cross-term pairs (s, s') and their gq/beta column j
PAIRS = [(0, 1), (1, 2), (2, 3), (0, 2), (1, 3), (0, 3)]

_CACHED = {}


def _emit_body(nc, pool, psum):
    xw_d = nc.dram_tensor("xw", [D, XWC], FP16, kind="ExternalInput")
    out_d = nc.dram_tensor("out", [1, 128, 1, B], FP16, kind="ExternalOutput")

    def st(shape, dtype, name, space_pool=None):
        sp_ = space_pool if space_pool is not None else pool
        return sp_.tile(shape, dtype, tag=name, name=name)

    def mm(out, lhsT, rhs, start, stop):
        return nc.tensor.matmul(out, lhsT, rhs, start=start, stop=stop,
                                skip_group_check=True)

    def dep(a, b, reason):
        bass._add_dep_helper(a.ins, b.ins, sync=False, reason=reason)

    def mm_chain(prev, out, lhsT, rhs, start, stop):
        i = mm(out, lhsT, rhs, start, stop)
        if prev is not None:
            dep(i, prev, "psum accumulation group order")
        return i

    # ------- PSUM tiles, one 2KB bank each (start zeroes a whole bank) ----
    Lq01 = st([CS, 2, B], F32, "Lq01", psum)     # bank0: L_0 | L_1
    Lq23 = st([CS, 2, B], F32, "Lq23", psum)     # bank1: L_2 | L_3
    nsqT = st([128, 512], F32, "nsqT", psum)     # bank2: nsq (cols 0:4)
    nsq = nsqT[0:CS, 0:S]
    gqT = st([128, 512], F32, "gqT", psum)       # bank3: gq (cols 0:6)
    gq = gqT[0:CS, 0:6]
    bankX = st([128, 512], F32, "bankX", psum)   # bank4: nx row | rnx_bc
    nx = bankX[0:1, 0:B]
    rnx_bc = bankX[0:CS, B:2 * B]
    numden = st([CS, 2, B], F32, "numden", psum)  # bank5: num | den
    numb = numden[:, 0, :]
    denb = numden[:, 1, :]

    def Lsl(s):
        return (Lq01 if s < 2 else Lq23)[:, s % 2, :]

    # ---------------- tiny init + act table warm ----------------
    warm = st([1, 1], F32, "warm")
    nc.vector.memset(warm[:], 1.0)
    warm3 = st([1, 1], F32, "warm3")
    nc.scalar.activation(warm3[:], warm[:], AF.Abs_reciprocal_sqrt)

    ones_w = st([128, 1], FP16, "ones_w")
    nc.vector.memset(ones_w[:], 1.0)
    ones_row = st([1, 128], FP16, "ones_row")
    nc.vector.memset(ones_row[:], 1.0)
    dum = st([128, 128], FP16, "dum")
    nc.vector.memset(dum[:], 0.03)
    eye = st([CS, CS], FP16, "eye")
    nc.vector.memset(eye[:], 1.0)
    ctx0 = st([128, 1], I32, "ctx0")
    nc.vector.memset(ctx0[:], 0)
    ot = st([128, 1, 1, B], FP16, "ot")
    nc.vector.memset(ot[:], 0.0)

    # ---------------- PE warmup (bank-5 openers, reset by num later) ------
    for _ in range(N_WARM):
        mm(numden[0:1, 0, 0:128], ones_w[:], dum[:], start=True, stop=True)

    # ---------------- input DMAs ----------------
    xw = st([128, KP, XWC], FP16, "xw")
    nc.gpsimd.dma_start(xw[:, 3, :], xw_d[3 * 128:4 * 128, :])
    for k in range(KP - 1):
        nc.sync.dma_start(xw[:, k, :], xw_d[k * 128:(k + 1) * 128, :])

    nc.gpsimd.affine_select(
        eye[:], eye[:], pattern=[[-1, CS]], compare_op=OP.is_equal,
        fill=0.0, base=0, channel_multiplier=1,
    )

    def wsl(k, lo, hi):
        return xw[:, k, B + lo:B + hi]

    # ---------------- per-k products (arrival order; last chunk on DVE) ---
    prodD = st([128, KP, R], FP16, "prodD")
    prodA = st([128, KP, 3 * CS], FP16, "prodA")   # (0,1),(1,2),(2,3)
    prodB = st([128, KP, 2 * CS], FP16, "prodB")   # (0,2),(1,3)
    prodC = st([128, KP, CS], FP16, "prodC")       # (0,3)
    xsq = st([128, KP, B], FP16, "xsq")
    for k in KORD:
        if k != KLAST:
            nc.scalar.square(prodD[:, k, :], wsl(k, 0, R))           # Act
            nc.vector.tensor_tensor(prodA[:, k, :], wsl(k, 0, 3 * CS),
                                    wsl(k, CS, R), OP.mult)          # DVE
            nc.vector.tensor_tensor(prodB[:, k, :], wsl(k, 0, 2 * CS),
                                    wsl(k, 2 * CS, R), OP.mult)      # DVE
        else:
            nc.vector.tensor_tensor(prodD[:, k, :], wsl(k, 0, R),
                                    wsl(k, 0, R), OP.mult)           # DVE
            nc.vector.tensor_tensor(prodA[:, k, :], wsl(k, 0, 3 * CS),
                                    wsl(k, CS, R), OP.mult)
            nc.vector.tensor_tensor(prodB[:, k, :], wsl(k, 0, 2 * CS),
                                    wsl(k, 2 * CS, R), OP.mult)
        nc.gpsimd.tensor_tensor(prodC[:, k, :], wsl(k, 0, CS),
                                wsl(k, 3 * CS, R), OP.mult)          # Pool
        nc.gpsimd.tensor_tensor(xsq[:, k, :], xw[:, k, 0:B],
                                xw[:, k, 0:B], OP.mult)              # Pool

    # ---------------- PE: L matmuls + reductions ----------------
    Lmm = [None] * S
    Nmm = [None] * S
    Gmm = [None] * 6
    nxmm = None
    nopener = None  # nsq bank opener
    gopener = None  # gq bank opener
    xopener = None  # bank-X opener

    def emit_reductions(ki, k):
        nonlocal nxmm, nopener, gopener, xopener
        first = ki == 0
        for s in range(S):
            Nmm[s] = mm_chain(Nmm[s], nsqT[0:CS, s:s + 1],
                              prodD[:, k, s * CS:(s + 1) * CS], ones_w[:],
                              start=(first and s == 0), stop=(ki == KP - 1))
            if nopener is None:
                nopener = Nmm[0]
            elif first and s > 0:
                dep(Nmm[s], nopener, "bank opener first")
        for j in range(3):
            Gmm[j] = mm_chain(Gmm[j], gqT[0:CS, j:j + 1],
                              prodA[:, k, j * CS:(j + 1) * CS], ones_w[:],
                              start=(first and j == 0), stop=(ki == KP - 1))
            if gopener is None:
                gopener = Gmm[0]
            elif first and j > 0:
                dep(Gmm[j], gopener, "bank opener first")
        for j in range(2):
            Gmm[3 + j] = mm_chain(Gmm[3 + j], gqT[0:CS, 3 + j:4 + j],
                                  prodB[:, k, j * CS:(j + 1) * CS], ones_w[:],
                                  start=False, stop=(ki == KP - 1))
            if first:
                dep(Gmm[3 + j], gopener, "bank opener first")
        Gmm[5] = mm_chain(Gmm[5], gqT[0:CS, 5:6],
                          prodC[:, k, :], ones_w[:], start=False,
                          stop=(ki == KP - 1))
        if first:
            dep(Gmm[5], gopener, "bank opener first")
        nxmm = mm_chain(nxmm, nx, ones_w[:], xsq[:, k, :],
                        start=first, stop=(ki == KP - 1))
        if first:
            xopener = nxmm

    for ki, k in enumerate(KORD[:-1]):
        for s in range(S):
            Lmm[s] = mm_chain(Lmm[s], Lsl(s), wsl(k, s * CS, (s + 1) * CS),
                              xw[:, k, 0:B],
                              start=(ki == 0 and s % 2 == 0), stop=False)
            if ki == 0 and s == 1:
                dep(Lmm[1], Lmm[0], "bank01 opener first")
            if ki == 0 and s == 3:
                dep(Lmm[3], Lmm[2], "bank23 opener first")
        emit_reductions(ki, k)

    # last chunk: L s0,s1 first, then nsq closers, then L s2,s3, then rest.
    k = KLAST
    for s in (0, 1):
        Lmm[s] = mm_chain(Lmm[s], Lsl(s), wsl(k, s * CS, (s + 1) * CS),
                          xw[:, k, 0:B], start=False, stop=True)
    for s in range(S):
        Nmm[s] = mm_chain(Nmm[s], nsqT[0:CS, s:s + 1],
                          prodD[:, k, s * CS:(s + 1) * CS], ones_w[:],
                          start=False, stop=True)
    for s in (2, 3):
        Lmm[s] = mm_chain(Lmm[s], Lsl(s), wsl(k, s * CS, (s + 1) * CS),
                          xw[:, k, 0:B], start=False, stop=True)
    for j in range(3):
        Gmm[j] = mm_chain(Gmm[j], gqT[0:CS, j:j + 1],
                          prodA[:, k, j * CS:(j + 1) * CS], ones_w[:],
                          start=False, stop=True)
    for j in range(2):
        Gmm[3 + j] = mm_chain(Gmm[3 + j], gqT[0:CS, 3 + j:4 + j],
                              prodB[:, k, j * CS:(j + 1) * CS], ones_w[:],
                              start=False, stop=True)
    Gmm[5] = mm_chain(Gmm[5], gqT[0:CS, 5:6], prodC[:, k, :],
                      ones_w[:], start=False, stop=True)
    nxmm = mm_chain(nxmm, nx, ones_w[:], xsq[:, k, :], start=False, stop=True)

    # ---------------- L PSUM -> SBUF fp16 (s0,s1 half on DVE) -----------
    mh01 = st([CS, 2, B], FP16, "mh01")
    nc.vector.tensor_scalar_add(mh01[:, :, :], Lq01[:, :, :], 0.0)   # DVE

    def ms(s):
        assert s < 2
        return mh01[:, s, :]

    # ---------------- coefficients ----------------
    alpha = st([CS, S], F32, "alpha")
    nc.vector.reciprocal(alpha[:], nsq)                              # DVE
    t6 = st([CS, 6], F32, "t6")
    nc.vector.tensor_tensor(t6[:, 0:3], alpha[:, 0:3], alpha[:, 1:4], OP.mult)
    nc.vector.tensor_tensor(t6[:, 3:5], alpha[:, 0:2], alpha[:, 2:4], OP.mult)
    nc.vector.tensor_tensor(t6[:, 5:6], alpha[:, 0:1], alpha[:, 3:4], OP.mult)
    beta = st([CS, 6], F32, "beta")
    nc.vector.scalar_tensor_tensor(
        out=beta[:], in0=gq, scalar=2.0, in1=t6[:], op0=OP.mult, op1=OP.mult)

    # rnw for Act's scaled squares (Act, off DVE)
    rnw = st([CS, S], F32, "rnw")
    nc.scalar.activation(rnw[:], nsq, AF.Abs_reciprocal_sqrt)        # Act

    # mh23 half on Act (after rnw so rnw lands early)
    mh23 = st([CS, 2, B], FP16, "mh23")
    nc.scalar.copy(mh23[:, :, :], Lq23[:, :, :])                     # Act

    def ms23(s):
        assert s >= 2
        return mh23[:, s - 2, :]

    def msx(s):
        return ms(s) if s < 2 else ms23(s)

    # ---------------- scaled quadratic terms ----------------
    # spn: P'_s = alpha_s L_s^2 ; spd: C'_j = beta_j L_s L_s'
    spn = st([CS, S, B], FP16, "spn")
    spd = st([CS, 6, B], FP16, "spd")

    # P'_2, P'_3 on Act straight from PSUM: Square(rnw_s * L_s)
    nc.scalar.activation(spn[:, 2, :], Lsl(2), AF.Square,
                         scale=rnw[:, 2:3])
    nc.scalar.activation(spn[:, 3, :], Lsl(3), AF.Square,
                         scale=rnw[:, 3:4])

    # DVE: Q01 square, its two alpha-scales, packed pair products + scales
    q01 = st([CS, 2, B], FP16, "q01")
    nc.vector.tensor_tensor(q01[:, :, :], mh01[:, :, :], mh01[:, :, :],
                            OP.mult)
    nc.vector.tensor_scalar_mul(spn[:, 0, :], q01[:, 0, :], alpha[:, 0:1])
    nc.vector.tensor_scalar_mul(spn[:, 1, :], q01[:, 1, :], alpha[:, 1:2])

    psA = st([CS, 3, B], FP16, "psA")   # (0,1),(1,2),(2,3) unscaled
    # in0 = m0,m1,m2 ; in1 = m1,m2,m3: m2,m3 live in mh23 — two packed ops
    nc.vector.tensor_tensor(psA[:, 0, :], ms(0), ms(1), OP.mult)
    nc.vector.tensor_scalar_mul(spd[:, 0, :], psA[:, 0, :], beta[:, 0:1])
    nc.vector.tensor_tensor(psA[:, 1, :], ms(1), ms23(2), OP.mult)
    nc.vector.tensor_scalar_mul(spd[:, 1, :], psA[:, 1, :], beta[:, 1:2])
    nc.vector.tensor_tensor(psA[:, 2, :], ms23(2), ms23(3), OP.mult)
    nc.vector.tensor_scalar_mul(spd[:, 2, :], psA[:, 2, :], beta[:, 2:3])
    psB = st([CS, 2, B], FP16, "psB")   # (0,2),(1,3)
    nc.vector.tensor_tensor(psB[:, 0, :], ms(0), ms23(2), OP.mult)
    nc.vector.tensor_scalar_mul(spd[:, 3, :], psB[:, 0, :], beta[:, 3:4])
    nc.vector.tensor_tensor(psB[:, 1, :], ms(1), ms23(3), OP.mult)
    nc.vector.tensor_scalar_mul(spd[:, 4, :], psB[:, 1, :], beta[:, 4:5])
    psC = st([CS, B], FP16, "psC")      # (0,3)
    nc.vector.tensor_tensor(psC[:], ms(0), ms23(3), OP.mult)
    nc.vector.tensor_scalar_mul(spd[:, 5, :], psC[:], beta[:, 5:6])

    # ---------------- rnx path (Act + PE, off critical) ----------------
    rnx_row = st([1, B], FP16, "rnx_row")
    nc.scalar.activation(rnx_row[:], nx, AF.Abs_reciprocal_sqrt)

    # ---------------- num/den accumulation on PE (identity matmuls) -----
    nmm = mm(numb, eye[:], spn[:, 0, :], start=True, stop=False)
    dopener = nmm
    dmm = mm(denb, eye[:], spn[:, 0, :], start=False, stop=False)
    dep(dmm, dopener, "bank opener first")
    # order roughly by expected availability
    order = [("n", 1), ("d", 1), ("n", 2), ("d", 2), ("n", 3), ("d", 3),
             ("d", 4 + 0), ("d", 4 + 1), ("d", 4 + 2), ("d", 4 + 3),
             ("d", 4 + 4), ("d", 4 + 5)]
    rbc = None
    for which, idx in order:
        if which == "n":
            nmm = mm_chain(nmm, numb, eye[:], spn[:, idx, :],
                           start=False, stop=(idx == 3))
        elif idx < 4:
            dmm = mm_chain(dmm, denb, eye[:], spn[:, idx, :],
                           start=False, stop=False)
        else:
            j = idx - 4
            dmm = mm_chain(dmm, denb, eye[:], spd[:, j, :],
                           start=False, stop=(j == 5))
        if which == "n" and idx == 3 and rbc is None:
            # start=True: opens its own zero region over partitions 0:125
            # (nx's opener covered partition 0 only). Safe to re-zero the
            # bank here: nx was already consumed by rnx_row, which this
            # matmul reads.
            rbc = mm(rnx_bc, ones_row[:, 0:CS], rnx_row[:],
                     start=True, stop=True)
            dep(rbc, xopener, "bank opener first")

    # ---------------- tail ----------------
    rnx_sb = st([CS, B], FP16, "rnx_sb")
    nc.scalar.copy(rnx_sb[:], rnx_bc)
    u = st([CS, B], FP16, "u")
    nc.vector.tensor_tensor(u[:], numb, rnx_sb[:], OP.mult)
    srd = st([CS, B], FP16, "srd")
    nc.scalar.activation(srd[:], denb, AF.Abs_reciprocal_sqrt)
    nc.vector.tensor_tensor(ot[0:CS, 0, 0, :], u[:], srd[:], OP.mult)

    # output: SWDGE descriptors prepared after the ot write (BIRSim reads
    # the source at prep position), fired by trigger_dma.
    sem_ot = nc.alloc_semaphore("dma_ot")
    nc.gpsimd.kv_writeback(
        out_d[:, :, :, :], ot[:, :, :, :], ctx0[:],
        prepare_only=True, sem=sem_ot,
    )
    nc.gpsimd.trigger_dma(count=None)


def _build_nc():
    nc = bacc.Bacc(
        "TRN2",
        target_bir_lowering=False,
        debug=False,
        enable_asserts=False,
        num_devices=NCORES,
    )
    with tile.TileContext(nc) as tc:
        with (
            tc.tile_pool(name="psum", bufs=1, space="PSUM") as psum,
            tc.tile_pool(name="main", bufs=1) as pool,
        ):
            _emit_body(nc, pool, psum)
    nc.compile()
    _fix_swdge_waits(nc)
    return nc


def _fix_swdge_waits(nc):
    """Point consumer waits at the sems the SWDGE descriptors actually fire.

    Tile routes data deps on prepared-SWDGE outputs through per-lane DMASW<i>
    semaphores, but the hardware descriptor encodes exactly one sem — the
    user sem passed via ``sem=`` (on_update[0], +16 at transfer end). Tile
    never attaches the DMASW increment for gen_mode==1 preps, leaving those
    DMASW waits unsatisfiable. Rewrite each unsatisfied ``DMASW<i> >= 16``
    wait to the user sem of the prep on that lane (lanes assigned
    round-robin over Pool DMA instructions in program order).
    """
    import re

    fn = nc.m.functions[0]
    lane_sem = {}
    updated = set()
    n_dma = 0
    for blk in fn.blocks:
        for ins in blk.instructions:
            if ins.sync_info is None:
                continue
            for upd in ins.sync_info.on_update:
                if upd.ant_name:
                    updated.add(upd.ant_name)
            if ins.engine == mybir.EngineType.Pool and (
                    type(ins).__name__ in ("InstDMACopy", "InstDMAGatherAnt",
                                           "InstKVWritebackAnt",
                                           "InstDMAScatterAddAnt",
                                           "InstPagedWritebackAnt")):
                if getattr(ins, "gen_mode", 0) == 1:
                    lane_sem[n_dma] = ins.sync_info.on_update[0]
                n_dma += 1
    for blk in fn.blocks:
        for ins in blk.instructions:
            if ins.sync_info is None:
                continue
            for w in ins.sync_info.on_wait:
                m = re.match(r"DMASW(\d+)_", w.ant_name or "")
                if not m or w.ant_name in updated:
                    continue
                lane = int(m.group(1))
                assert w.wait_value == 16, (ins.name, w.ant_name, w.wait_value)
                assert lane in lane_sem, (ins.name, w.ant_name, lane_sem)
                u = lane_sem[lane]
                w.id = u.id
                w.ant_name = u.ant_name


def _get_nc():
    if "nc" not in _CACHED:
        _CACHED["nc"] = _build_nc()
    return _CACHED["nc"]


def _make_in_maps(x, W):
    x = np.ascontiguousarray(np.asarray(x, dtype=np.float32))
    W = np.ascontiguousarray(np.asarray(W, dtype=np.float32))
    xT = x.T.astype(np.float16)  # [D, B]
    in_maps = []
    for i in range(NCORES):
        Ws = W[i * CS:(i + 1) * CS].astype(np.float16)      # [CS, S, D]
        wT = Ws.transpose(2, 1, 0).reshape(D, R)            # [D, s*CS+c]
        xw = np.ascontiguousarray(np.concatenate([xT, wT], axis=1))
        in_maps.append({"xw": xw})
    return in_maps


def run(x, W, trace=False):
    nc = _get_nc()
    in_maps = _make_in_maps(x, W)
    res = run_bass_kernel_spmd(
        nc, in_maps, core_ids=list(range(NCORES)), trace=trace
    )
    shards = []
    for i in range(NCORES):
        o = np.asarray(res.results[i]["out"]).reshape(128, B)[0:CS, :]
        shards.append(o.astype(np.float32))
    out = np.concatenate([s.T for s in shards], axis=1)  # [B, C]
    return np.ascontiguousarray(out.astype(np.float32)), res


def kernel(x, W):
    out, _ = run(x, W, trace=False)
    return out


# revision 24
# speedup vs baseline: 1.2073x; 1.1484x over previous
"""AdaProj kernel for 8 TRN2 NeuronCores (baseline schedule + SWDGE-
prepared output writeback).

Math: per class c, sample b:
  L_s[c,b] = W[c,s,:] . x[b,:]   (raw matmul)
  rnw[c,s] = 1/||W[c,s,:]||, rnx[b] = 1/||x[b]||
  m_s = rnw_s * L_s
  num = sum_s m_s^2
  den = num + sum_{s<s'} h_ss' * m_s * m_s',  h_ss' = 2*Graw_ss'*rnw_s*rnw_s'
  out[c,b] = rnx_b * num / sqrt(den)

Structure:
  - x and W packed into ONE dram tensor xw [D, 256+500] fp16, loaded in 4
    contiguous k-chunks so matmuls start on chunk 0 while later chunks
    stream.
  - PE warmed with dummy matmuls during the DMA wait (p-state ramp).
  - All sum-over-D reductions (W norms, Gram pairs) are PE matmuls with
    free-size-1 outputs: lhsT = product chunk [128, <=125], rhs = ones.
  - num and the cross term accumulate in PSUM via identity-matmuls
    (lhsT = I_125) over the fp16 product tiles, freeing DVE adds.
  - Per-k product work split: W-squares on Act, pair products on DVE,
    (0,3) pair on gpsimd; xsq on DVE.
  - Output via kv_writeback: SWDGE descriptors generated right after the
    final ot write (Pool), fired by trigger_dma — the transfer skips the
    HWDGE descriptor stage (625ns) and DGE ramp (650ns) on the tail.

Sharding: W split over classes C (125/core); x replicated; host
concatenates the per-core [125, 256] outputs.
"""

import numpy as np

import concourse.bacc as bacc
import concourse.bass as bass
import concourse.mybir as mybir
import concourse.tile as tile
from concourse.bass_utils import run_bass_kernel_spmd

B, C, S, D = 256, 1000, 4, 512
NCORES = 8
CS = C // NCORES          # 125 classes per core
R = CS * S                # 500 W rows per core
KP = D // 128             # 4 contraction chunks
XW = B + R                # 756 packed columns: [x | w]

F32 = mybir.dt.float32
FP16 = mybir.dt.float16
I32 = mybir.dt.int32
AF = mybir.ActivationFunctionType
OP = mybir.AluOpType

N_WARM = 22  # dummy PE matmuls holding the p-state ramp until data lands

_CACHED = {}


def _emit_body(nc, pool, psum):
    xw_d = nc.dram_tensor("xw", [D, XW], FP16, kind="ExternalInput")
    out_d = nc.dram_tensor("out", [1, 128, 1, B], FP16, kind="ExternalOutput")

    def st(shape, dtype, name, space_pool=None):
        sp = space_pool if space_pool is not None else pool
        return sp.tile(shape, dtype, tag=name, name=name)

    def mm(out, lhsT, rhs, start, stop):
        return nc.tensor.matmul(out, lhsT, rhs, start=start, stop=stop,
                                skip_group_check=True)

    def mm_chain(prev, out, lhsT, rhs, start, stop):
        i = mm(out, lhsT, rhs, start, stop)
        if prev is not None:
            bass._add_dep_helper(i.ins, prev.ins, sync=False,
                                 reason="psum accumulation group order")
        return i

    # ---------------- tiny init + act table warm ----------------
    warm = st([1, 1], F32, "warm")
    nc.vector.memset(warm[:], 1.0)
    warm3 = st([1, 1], F32, "warm3")
    nc.scalar.activation(warm3[:], warm[:], AF.Abs_reciprocal_sqrt)

    ones_w = st([128, 1], FP16, "ones_w")
    nc.vector.memset(ones_w[:], 1.0)
    dum = st([128, 128], FP16, "dum")
    nc.vector.memset(dum[:], 0.03)
    ones_row = st([1, 128], FP16, "ones_row")
    nc.vector.memset(ones_row[:], 1.0)
    # identity [125,125] fp16 for the PSUM-accumulating identity matmuls
    eye = st([CS, CS], FP16, "eye")
    nc.vector.memset(eye[:], 1.0)
    nc.gpsimd.affine_select(
        eye[:], eye[:], pattern=[[-1, CS]], compare_op=OP.is_equal,
        fill=0.0, base=0, channel_multiplier=1,
    )
    ctx0 = st([128, 1], I32, "ctx0")
    nc.vector.memset(ctx0[:], 0)
    ot = st([128, 1, 1, B], FP16, "ot")
    nc.vector.memset(ot[:], 0.0)

    # ---------------- PE warmup (writes the numb bank, reset later) -----
    numb = st([CS, B], F32, "numb", psum)
    nx = numb[0:1, :]
    for i in range(N_WARM):
        mm(nx[:, 0:128], ones_w[:], dum[:], start=True, stop=True)

    # ---------------- input DMAs (SP HWDGE, 4 k-chunks) ----------------
    xw = st([128, KP, XW], FP16, "xw")
    nc.gpsimd.dma_start(xw[:, 3, :], xw_d[3 * 128:4 * 128, :])
    for k in range(KP - 1):
        nc.sync.dma_start(xw[:, k, :], xw_d[k * 128:(k + 1) * 128, :])

    def wsl(k, lo, hi):
        return xw[:, k, B + lo:B + hi]

    # ---------------- PE: L matmuls, k-major (packed banks) -------------
    Lp = [st([CS, B], F32, f"L{s}", psum) for s in range(S)]
    Lsl = [Lp[s][:] for s in range(S)]
    KORD = [0, 3, 1, 2]
    Lmm = [None] * S
    for ki, k in enumerate(KORD):
        for s in range(S):
            Lmm[s] = mm_chain(Lmm[s], Lsl[s], wsl(k, s * CS, (s + 1) * CS),
                              xw[:, k, 0:B], start=(ki == 0), stop=(ki == KP - 1))

    # ---------------- per-k products: Act squares, DVE pairs, Pool (0,3)
    prodD = st([128, KP, R], FP16, "prodD")
    prodA = st([128, KP, 3 * CS], FP16, "prodA")   # (0,1),(1,2),(2,3)
    prodB = st([128, KP, 2 * CS], FP16, "prodB")   # (0,2),(1,3)
    prodC = st([128, KP, CS], FP16, "prodC")       # (0,3)
    xsq = st([128, KP, B], FP16, "xsq")
    KLAST = KORD[-1]   # last-arriving chunk (k2)
    for k in KORD:
        if k != KLAST:
            nc.scalar.activation(prodD[:, k, :], wsl(k, 0, R), AF.Square)
            nc.vector.tensor_tensor(prodA[:, k, :], wsl(k, 0, 3 * CS), wsl(k, CS, R), OP.mult)
            nc.vector.tensor_tensor(prodB[:, k, :], wsl(k, 0, 2 * CS), wsl(k, 2 * CS, R), OP.mult)
            nc.gpsimd.tensor_tensor(xsq[:, k, :], xw[:, k, 0:B], xw[:, k, 0:B], OP.mult)
        else:
            nc.vector.tensor_tensor(prodD[:, k, 0:2 * CS], wsl(k, 0, 2 * CS), wsl(k, 0, 2 * CS), OP.mult)
            pd3b = nc.vector.tensor_tensor(prodD[:, k, 2 * CS:R], wsl(k, 2 * CS, R), wsl(k, 2 * CS, R), OP.mult)
        nc.gpsimd.tensor_tensor(prodC[:, k, :], wsl(k, 0, CS), wsl(k, 3 * CS, R), OP.mult)

    # ---------------- PE: norm/gram reductions (free-size-1 matmuls) ----
    nsqx = st([CS, 8 + B], F32, "nsqx", psum)
    nsq = nsqx[:, 0:S]
    gqx = st([CS, 6], F32, "gqx", psum)
    gq = gqx[:, 0:6]
    Nmm = [None] * S
    for ki, k in enumerate(KORD):
        for s in range(S):
            Nmm[s] = mm_chain(Nmm[s], nsqx[:, s:s + 1],
                              prodD[:, k, s * CS:(s + 1) * CS], ones_w[:],
                              start=(ki == 0 and s == 0), stop=(ki == KP - 1))
            if ki == 0 and s > 0:
                bass._add_dep_helper(Nmm[s].ins, Nmm[0].ins, sync=False,
                                     reason="bank opener first")
    last_nsq = Nmm[S - 1]
    # ---------------- rnw + m copies ----------------
    rnw = st([CS, S], F32, "rnw")
    nc.scalar.activation(rnw[:], nsq, AF.Abs_reciprocal_sqrt)
    m = st([CS, S, B], FP16, "m")
    m0i = nc.scalar.mul(m[:, 0, :], Lsl[0], rnw[:, 0:1])
    nc.vector.tensor_scalar_mul(m[:, 1, :], Lsl[1], rnw[:, 1:2])
    m2i = nc.scalar.mul(m[:, 2, :], Lsl[2], rnw[:, 2:3])
    m3i = nc.vector.tensor_scalar_mul(m[:, 3, :], Lsl[3], rnw[:, 3:4])
    # deferred last-chunk products (gram + rnx inputs, non-critical; after prodD)
    pa3 = nc.vector.tensor_tensor(prodA[:, KLAST, :], wsl(KLAST, 0, 3 * CS), wsl(KLAST, CS, R), OP.mult)
    bass._add_dep_helper(pa3.ins, pd3b.ins, sync=False, reason="prodD first")
    nc.vector.tensor_tensor(prodB[:, KLAST, :], wsl(KLAST, 0, 2 * CS), wsl(KLAST, 2 * CS, R), OP.mult)
    nc.vector.tensor_tensor(xsq[:, KLAST, :], xw[:, KLAST, 0:B], xw[:, KLAST, 0:B], OP.mult)

    # gram + rnx reductions (need all-k products)
    Gmm = [None] * 6
    for ki, k in enumerate(KORD):
        for j in range(3):
            Gmm[j] = mm_chain(Gmm[j], gqx[:, j:j + 1],
                              prodA[:, k, j * CS:(j + 1) * CS], ones_w[:],
                              start=(ki == 0 and j == 0), stop=(ki == KP - 1))
            if ki == 0 and j > 0:
                bass._add_dep_helper(Gmm[j].ins, Gmm[0].ins, sync=False,
                                     reason="bank opener first")
        for j in range(2):
            Gmm[3 + j] = mm_chain(Gmm[3 + j], gqx[:, 3 + j:4 + j],
                                  prodB[:, k, j * CS:(j + 1) * CS], ones_w[:],
                                  start=False, stop=(ki == KP - 1))
            if ki == 0:
                bass._add_dep_helper(Gmm[3 + j].ins, Gmm[0].ins, sync=False,
                                     reason="bank opener first")
        Gmm[5] = mm_chain(Gmm[5], gqx[:, 5:6], prodC[:, k, :], ones_w[:],
                          start=False, stop=(ki == KP - 1))
        if ki == 0:
            bass._add_dep_helper(Gmm[5].ins, Gmm[0].ins, sync=False,
                                 reason="bank opener first")

    # ---------------- rnx reduction on PE (reuses warm bank) ------------
    nxi = None
    for k in range(KP):
        nxi = mm_chain(nxi, nx, ones_w[:], xsq[:, k, :],
                       start=(k == 0), stop=(k == KP - 1))
        if k == 0:
            bass._add_dep_helper(nxi.ins, last_nsq.ins, sync=False,
                                 reason="norm reductions first on PE")

    # rnx row + broadcast (ordering hint keeps it behind m0 on Act)
    rnx_row = st([1, B], FP16, "rnx_row")
    rri = nc.scalar.activation(rnx_row[:], nx, AF.Abs_reciprocal_sqrt)
    bass._add_dep_helper(rri.ins, m2i.ins, sync=False,
                         reason="m copies first on Act")
    rnx_bc = nsqx[:, 8:8 + B]
    bci = mm(rnx_bc, ones_row[:, 0:CS], rnx_row[:], start=False, stop=True)
    bass._add_dep_helper(bci.ins, Nmm[0].ins, sync=False,
                         reason="bank opener first")

    # ---------------- gram coefficients (gpsimd) ----------------
    t6 = st([CS, 6], F32, "t6")
    nc.vector.tensor_tensor(t6[:, 0:3], rnw[:, 0:3], rnw[:, 1:4], OP.mult)
    nc.vector.tensor_tensor(t6[:, 3:5], rnw[:, 0:2], rnw[:, 2:4], OP.mult)
    nc.vector.tensor_tensor(t6[:, 5:6], rnw[:, 0:1], rnw[:, 3:4], OP.mult)
    h = st([CS, 6], F32, "h")
    nc.vector.scalar_tensor_tensor(
        out=h[:], in0=gq, scalar=2.0, in1=t6[:], op0=OP.mult, op1=OP.mult,
    )

    # ---------------- epilogue products + scaled cross terms ------------
    Q01 = st([CS, 2, B], FP16, "Q01")
    nc.vector.tensor_tensor(Q01[:], m[:, 0:2, :], m[:, 0:2, :], OP.mult)
    Q23 = st([CS, 2, B], FP16, "Q23")
    nc.vector.tensor_tensor(Q23[:], m[:, 2:4, :], m[:, 2:4, :], OP.mult)
    psA = st([CS, 3, B], FP16, "psA")
    nc.vector.tensor_tensor(psA[:], m[:, 0:3, :], m[:, 1:4, :], OP.mult)
    cpA = st([CS, 3, B], FP16, "cpA")
    nc.vector.tensor_scalar_mul(cpA[:, 0, :], psA[:, 0, :], h[:, 0:1])
    nc.vector.tensor_scalar_mul(cpA[:, 1, :], psA[:, 1, :], h[:, 1:2])
    nc.scalar.mul(cpA[:, 2, :], psA[:, 2, :], h[:, 2:3])
    psB = st([CS, 2, B], FP16, "psB")
    nc.vector.tensor_tensor(psB[:], m[:, 0:2, :], m[:, 2:4, :], OP.mult)
    cpB = st([CS, 2, B], FP16, "cpB")
    nc.vector.tensor_scalar_mul(cpB[:, 0, :], psB[:, 0, :], h[:, 3:4])
    nc.vector.tensor_scalar_mul(cpB[:, 1, :], psB[:, 1, :], h[:, 4:5])
    # pair (0,3) on gpsimd
    psC = st([CS, B], FP16, "psC")
    nc.gpsimd.tensor_tensor(psC[:], m[:, 0, :], m[:, 3, :], OP.mult)
    cpC = st([CS, B], FP16, "cpC")
    nc.gpsimd.tensor_scalar_mul(cpC[:], psC[:], h[:, 5:6])

    # ---------------- num & den accumulation on PE (identity matmuls) ---
    Qsl = [Q01[:, 0, :], Q01[:, 1, :], Q23[:, 0, :], Q23[:, 1, :]]
    nmm = None
    for s in range(S):
        nmm = mm_chain(nmm, numb[:], eye[:], Qsl[s],
                       start=(s == 0), stop=(s == S - 1))
    denb = st([CS, B], F32, "denb", psum)
    dmm = None
    for s in range(S):
        dmm = mm_chain(dmm, denb[:], eye[:], Qsl[s],
                       start=(s == 0), stop=False)
    dmm = mm_chain(dmm, denb[:], eye[:], cpC[:], start=False, stop=False)
    for j in range(3):
        dmm = mm_chain(dmm, denb[:], eye[:], cpA[:, j, :], start=False, stop=False)
    for j in range(2):
        dmm = mm_chain(dmm, denb[:], eye[:], cpB[:, j, :], start=False, stop=(j == 1))

    # u = num * rnx (off critical path)
    rnx_sb = st([CS, B], FP16, "rnx_sb")
    nc.scalar.copy(rnx_sb[:], rnx_bc)
    u = st([CS, B], FP16, "u")
    nc.vector.tensor_tensor(u[:], numb[:], rnx_sb[:], OP.mult)

    srd = st([CS, B], FP16, "srd")
    nc.scalar.activation(srd[:], denb[:], AF.Abs_reciprocal_sqrt)
    nc.vector.tensor_tensor(ot[0:CS, 0, 0, :], u[:], srd[:], OP.mult)

    # output: SWDGE descriptors prepared after the ot write (the simulator
    # reads the source at prep position), fired by trigger_dma — skips the
    # HWDGE descriptor stage + DGE ramp on the critical tail.
    sem_ot = nc.alloc_semaphore("dma_ot")
    nc.gpsimd.kv_writeback(
        out_d[:, :, :, :], ot[:, :, :, :], ctx0[:],
        prepare_only=True, sem=sem_ot,
    )
    nc.gpsimd.trigger_dma(count=None)


def _build_nc():
    nc = bacc.Bacc(
        "TRN2",
        target_bir_lowering=False,
        debug=False,
        enable_asserts=False,
        num_devices=NCORES,
    )
    with tile.TileContext(nc) as tc:
        with (
            tc.tile_pool(name="main", bufs=1) as pool,
            tc.tile_pool(name="psum", bufs=1, space="PSUM") as psum,
        ):
            _emit_body(nc, pool, psum)
    nc.compile()
    _fix_swdge_waits(nc)
    return nc


def _fix_swdge_waits(nc):
    """Point consumer waits at the sems the SWDGE descriptors actually fire.

    Tile routes data deps on prepared-SWDGE outputs through per-lane DMASW<i>
    semaphores, but the hardware descriptor encodes exactly one sem — the
    user sem passed via ``sem=`` (on_update[0], +16 at transfer end). Tile
    never attaches the DMASW increment for gen_mode==1 preps, leaving those
    DMASW waits unsatisfiable. Rewrite each unsatisfied ``DMASW<i> >= 16``
    wait to the user sem of the prep on that lane (lanes assigned
    round-robin over Pool DMA instructions in program order).
    """
    import re

    fn = nc.m.functions[0]
    lane_sem = {}
    updated = set()
    n_dma = 0
    for blk in fn.blocks:
        for ins in blk.instructions:
            if ins.sync_info is None:
                continue
            for upd in ins.sync_info.on_update:
                if upd.ant_name:
                    updated.add(upd.ant_name)
            if ins.engine == mybir.EngineType.Pool and (
                    type(ins).__name__ in ("InstDMACopy", "InstDMAGatherAnt",
                                           "InstKVWritebackAnt",
                                           "InstDMAScatterAddAnt",
                                           "InstPagedWritebackAnt")):
                if getattr(ins, "gen_mode", 0) == 1:
                    lane_sem[n_dma] = ins.sync_info.on_update[0]
                n_dma += 1
    for blk in fn.blocks:
        for ins in blk.instructions:
            if ins.sync_info is None:
                continue
            for w in ins.sync_info.on_wait:
                m = re.match(r"DMASW(\d+)_", w.ant_name or "")
                if not m or w.ant_name in updated:
                    continue
                lane = int(m.group(1))
                assert w.wait_value == 16, (ins.name, w.ant_name, w.wait_value)
                assert lane in lane_sem, (ins.name, w.ant_name, lane_sem)
                u = lane_sem[lane]
                w.id = u.id
                w.ant_name = u.ant_name


def _get_nc():
    if "nc" not in _CACHED:
        _CACHED["nc"] = _build_nc()
    return _CACHED["nc"]


def _make_in_maps(x, W):
    x = np.ascontiguousarray(np.asarray(x, dtype=np.float32))
    W = np.ascontiguousarray(np.asarray(W, dtype=np.float32))
    xT = x.T.astype(np.float16)  # [D, B]
    in_maps = []
    for i in range(NCORES):
        Ws = W[i * CS:(i + 1) * CS].astype(np.float16)      # [CS, S, D]
        wT = Ws.transpose(2, 1, 0).reshape(D, R)            # [D, s*CS+c]
        xw = np.ascontiguousarray(np.concatenate([xT, wT], axis=1))
        in_maps.append({"xw": xw})
    return in_maps


def run(x, W, trace=False):
    nc = _get_nc()
    in_maps = _make_in_maps(x, W)
    res = run_bass_kernel_spmd(
        nc, in_maps, core_ids=list(range(NCORES)), trace=trace
    )
    shards = []
    for i in range(NCORES):
        o = np.asarray(res.results[i]["out"]).reshape(128, B)[0:CS, :]
        shards.append(o.astype(np.float32))
    out = np.concatenate([s.T for s in shards], axis=1)  # [B, C]
    return np.ascontiguousarray(out.astype(np.float32)), res


def kernel(x, W):
    out, _ = run(x, W, trace=False)
    return out


# revision 25
# speedup vs baseline: 1.2139x; 1.0055x over previous
"""AdaProj kernel for 8 TRN2 NeuronCores (baseline schedule + SWDGE-
prepared output writeback).

Math: per class c, sample b:
  L_s[c,b] = W[c,s,:] . x[b,:]   (raw matmul)
  rnw[c,s] = 1/||W[c,s,:]||, rnx[b] = 1/||x[b]||
  m_s = rnw_s * L_s
  num = sum_s m_s^2
  den = num + sum_{s<s'} h_ss' * m_s * m_s',  h_ss' = 2*Graw_ss'*rnw_s*rnw_s'
  out[c,b] = rnx_b * num / sqrt(den)

Structure:
  - x and W packed into ONE dram tensor xw [D, 256+500] fp16, loaded in 4
    contiguous k-chunks so matmuls start on chunk 0 while later chunks
    stream.
  - PE warmed with dummy matmuls during the DMA wait (p-state ramp).
  - All sum-over-D reductions (W norms, Gram pairs) are PE matmuls with
    free-size-1 outputs: lhsT = product chunk [128, <=125], rhs = ones.
  - num and the cross term accumulate in PSUM via identity-matmuls
    (lhsT = I_125) over the fp16 product tiles, freeing DVE adds.
  - Per-k product work split: W-squares on Act, pair products on DVE,
    (0,3) pair on gpsimd; xsq on DVE.
  - Output via kv_writeback: SWDGE descriptors generated right after the
    final ot write (Pool), fired by trigger_dma — the transfer skips the
    HWDGE descriptor stage (625ns) and DGE ramp (650ns) on the tail.

Sharding: W split over classes C (125/core); x replicated; host
concatenates the per-core [125, 256] outputs.
"""

import numpy as np

import concourse.bacc as bacc
import concourse.bass as bass
import concourse.mybir as mybir
import concourse.tile as tile
from concourse.bass_utils import run_bass_kernel_spmd

B, C, S, D = 256, 1000, 4, 512
NCORES = 8
CS = C // NCORES          # 125 classes per core
R = CS * S                # 500 W rows per core
KP = D // 128             # 4 contraction chunks
XW = B + R                # 756 packed columns: [x | w]

F32 = mybir.dt.float32
FP16 = mybir.dt.float16
I32 = mybir.dt.int32
AF = mybir.ActivationFunctionType
OP = mybir.AluOpType

N_WARM = 20  # dummy PE matmuls holding the p-state ramp until data lands

_CACHED = {}


def _emit_body(nc, pool, psum):
    xw_d = nc.dram_tensor("xw", [D, XW], FP16, kind="ExternalInput")
    out_d = nc.dram_tensor("out", [1, 128, 1, B], FP16, kind="ExternalOutput")

    def st(shape, dtype, name, space_pool=None):
        sp = space_pool if space_pool is not None else pool
        return sp.tile(shape, dtype, tag=name, name=name)

    def mm(out, lhsT, rhs, start, stop):
        return nc.tensor.matmul(out, lhsT, rhs, start=start, stop=stop,
                                skip_group_check=True)

    def mm_chain(prev, out, lhsT, rhs, start, stop):
        i = mm(out, lhsT, rhs, start, stop)
        if prev is not None:
            bass._add_dep_helper(i.ins, prev.ins, sync=False,
                                 reason="psum accumulation group order")
        return i

    # ---------------- tiny init + act table warm ----------------
    warm = st([1, 1], F32, "warm")
    nc.vector.memset(warm[:], 1.0)
    warm3 = st([1, 1], F32, "warm3")
    nc.scalar.activation(warm3[:], warm[:], AF.Abs_reciprocal_sqrt)

    ones_w = st([128, 1], FP16, "ones_w")
    nc.vector.memset(ones_w[:], 1.0)
    dum = st([128, 128], FP16, "dum")
    nc.vector.memset(dum[:], 0.03)
    ones_row = st([1, 128], FP16, "ones_row")
    nc.vector.memset(ones_row[:], 1.0)
    # identity [125,125] fp16 for the PSUM-accumulating identity matmuls
    eye = st([CS, CS], FP16, "eye")
    nc.vector.memset(eye[:], 1.0)
    nc.gpsimd.affine_select(
        eye[:], eye[:], pattern=[[-1, CS]], compare_op=OP.is_equal,
        fill=0.0, base=0, channel_multiplier=1,
    )
    ctx0 = st([128, 1], I32, "ctx0")
    nc.vector.memset(ctx0[:], 0)
    ot = st([128, 1, 1, B], FP16, "ot")
    nc.vector.memset(ot[:], 0.0)

    # ---------------- PE warmup (writes the numb bank, reset later) -----
    numb = st([CS, B], F32, "numb", psum)
    nx = numb[0:1, :]
    for i in range(N_WARM):
        mm(nx[:, 0:128], ones_w[:], dum[:], start=True, stop=True)

    # ---------------- input DMAs (SP HWDGE, 4 k-chunks) ----------------
    xw = st([128, KP, XW], FP16, "xw")
    nc.gpsimd.dma_start(xw[:, 3, :], xw_d[3 * 128:4 * 128, :])
    for k in range(KP - 1):
        nc.sync.dma_start(xw[:, k, :], xw_d[k * 128:(k + 1) * 128, :])

    def wsl(k, lo, hi):
        return xw[:, k, B + lo:B + hi]

    # ---------------- PE: L matmuls, k-major (packed banks) -------------
    Lp = [st([CS, B], F32, f"L{s}", psum) for s in range(S)]
    Lsl = [Lp[s][:] for s in range(S)]
    KORD = [0, 3, 1, 2]
    Lmm = [None] * S
    for ki, k in enumerate(KORD):
        for s in range(S):
            Lmm[s] = mm_chain(Lmm[s], Lsl[s], wsl(k, s * CS, (s + 1) * CS),
                              xw[:, k, 0:B], start=(ki == 0), stop=(ki == KP - 1))

    # ---------------- per-k products: Act squares, DVE pairs, Pool (0,3)
    prodD = st([128, KP, R], FP16, "prodD")
    prodA = st([128, KP, 3 * CS], FP16, "prodA")   # (0,1),(1,2),(2,3)
    prodB = st([128, KP, 2 * CS], FP16, "prodB")   # (0,2),(1,3)
    prodC = st([128, KP, CS], FP16, "prodC")       # (0,3)
    xsq = st([128, KP, B], FP16, "xsq")
    KLAST = KORD[-1]   # last-arriving chunk (k2)
    for k in KORD:
        if k != KLAST:
            nc.scalar.activation(prodD[:, k, :], wsl(k, 0, R), AF.Square)
            nc.vector.tensor_tensor(prodA[:, k, :], wsl(k, 0, 3 * CS), wsl(k, CS, R), OP.mult)
            nc.vector.tensor_tensor(prodB[:, k, :], wsl(k, 0, 2 * CS), wsl(k, 2 * CS, R), OP.mult)
            nc.gpsimd.tensor_tensor(xsq[:, k, :], xw[:, k, 0:B], xw[:, k, 0:B], OP.mult)
        else:
            pd3b = nc.vector.tensor_tensor(prodD[:, k, :], wsl(k, 0, R), wsl(k, 0, R), OP.mult)
        nc.gpsimd.tensor_tensor(prodC[:, k, :], wsl(k, 0, CS), wsl(k, 3 * CS, R), OP.mult)

    # ---------------- PE: norm/gram reductions (free-size-1 matmuls) ----
    nsqx = st([CS, 8 + B], F32, "nsqx", psum)
    nsq = nsqx[:, 0:S]
    gqx = st([CS, 6], F32, "gqx", psum)
    gq = gqx[:, 0:6]
    Nmm = [None] * S
    for ki, k in enumerate(KORD):
        for s in range(S):
            Nmm[s] = mm_chain(Nmm[s], nsqx[:, s:s + 1],
                              prodD[:, k, s * CS:(s + 1) * CS], ones_w[:],
                              start=(ki == 0 and s == 0), stop=(ki == KP - 1))
            if ki == 0 and s > 0:
                bass._add_dep_helper(Nmm[s].ins, Nmm[0].ins, sync=False,
                                     reason="bank opener first")
    last_nsq = Nmm[S - 1]
    # ---------------- rnw + m copies ----------------
    rnw = st([CS, S], F32, "rnw")
    nc.scalar.activation(rnw[:], nsq, AF.Abs_reciprocal_sqrt)
    m = st([CS, S, B], FP16, "m")
    m0i = nc.scalar.mul(m[:, 0, :], Lsl[0], rnw[:, 0:1])
    nc.vector.tensor_scalar_mul(m[:, 1, :], Lsl[1], rnw[:, 1:2])
    m2i = nc.scalar.mul(m[:, 2, :], Lsl[2], rnw[:, 2:3])
    m3i = nc.vector.tensor_scalar_mul(m[:, 3, :], Lsl[3], rnw[:, 3:4])
    # deferred last-chunk products (gram + rnx inputs, non-critical; after prodD)
    pa3 = nc.vector.tensor_tensor(prodA[:, KLAST, :], wsl(KLAST, 0, 3 * CS), wsl(KLAST, CS, R), OP.mult)
    bass._add_dep_helper(pa3.ins, pd3b.ins, sync=False, reason="prodD first")
    nc.vector.tensor_tensor(prodB[:, KLAST, :], wsl(KLAST, 0, 2 * CS), wsl(KLAST, 2 * CS, R), OP.mult)
    nc.vector.tensor_tensor(xsq[:, KLAST, :], xw[:, KLAST, 0:B], xw[:, KLAST, 0:B], OP.mult)

    # gram + rnx reductions (need all-k products)
    Gmm = [None] * 6
    for ki, k in enumerate(KORD):
        for j in range(3):
            Gmm[j] = mm_chain(Gmm[j], gqx[:, j:j + 1],
                              prodA[:, k, j * CS:(j + 1) * CS], ones_w[:],
                              start=(ki == 0 and j == 0), stop=(ki == KP - 1))
            if ki == 0 and j > 0:
                bass._add_dep_helper(Gmm[j].ins, Gmm[0].ins, sync=False,
                                     reason="bank opener first")
        for j in range(2):
            Gmm[3 + j] = mm_chain(Gmm[3 + j], gqx[:, 3 + j:4 + j],
                                  prodB[:, k, j * CS:(j + 1) * CS], ones_w[:],
                                  start=False, stop=(ki == KP - 1))
            if ki == 0:
                bass._add_dep_helper(Gmm[3 + j].ins, Gmm[0].ins, sync=False,
                                     reason="bank opener first")
        Gmm[5] = mm_chain(Gmm[5], gqx[:, 5:6], prodC[:, k, :], ones_w[:],
                          start=False, stop=(ki == KP - 1))
        if ki == 0:
            bass._add_dep_helper(Gmm[5].ins, Gmm[0].ins, sync=False,
                                 reason="bank opener first")

    # ---------------- rnx reduction on PE (reuses warm bank) ------------
    nxi = None
    for k in range(KP):
        nxi = mm_chain(nxi, nx, ones_w[:], xsq[:, k, :],
                       start=(k == 0), stop=(k == KP - 1))
        if k == 0:
            bass._add_dep_helper(nxi.ins, last_nsq.ins, sync=False,
                                 reason="norm reductions first on PE")

    # rnx row + broadcast (ordering hint keeps it behind m0 on Act)
    rnx_row = st([1, B], FP16, "rnx_row")
    rri = nc.scalar.activation(rnx_row[:], nx, AF.Abs_reciprocal_sqrt)
    bass._add_dep_helper(rri.ins, m2i.ins, sync=False,
                         reason="m copies first on Act")
    rnx_bc = nsqx[:, 8:8 + B]
    bci = mm(rnx_bc, ones_row[:, 0:CS], rnx_row[:], start=False, stop=True)
    bass._add_dep_helper(bci.ins, Nmm[0].ins, sync=False,
                         reason="bank opener first")

    # ---------------- gram coefficients (gpsimd) ----------------
    t6 = st([CS, 6], F32, "t6")
    nc.vector.tensor_tensor(t6[:, 0:3], rnw[:, 0:3], rnw[:, 1:4], OP.mult)
    nc.vector.tensor_tensor(t6[:, 3:5], rnw[:, 0:2], rnw[:, 2:4], OP.mult)
    nc.vector.tensor_tensor(t6[:, 5:6], rnw[:, 0:1], rnw[:, 3:4], OP.mult)
    h = st([CS, 6], F32, "h")
    nc.vector.scalar_tensor_tensor(
        out=h[:], in0=gq, scalar=2.0, in1=t6[:], op0=OP.mult, op1=OP.mult,
    )

    # ---------------- epilogue products + scaled cross terms ------------
    Q01 = st([CS, 2, B], FP16, "Q01")
    nc.vector.tensor_tensor(Q01[:], m[:, 0:2, :], m[:, 0:2, :], OP.mult)
    Q23 = st([CS, 2, B], FP16, "Q23")
    nc.vector.tensor_tensor(Q23[:], m[:, 2:4, :], m[:, 2:4, :], OP.mult)
    psA = st([CS, 3, B], FP16, "psA")
    nc.vector.tensor_tensor(psA[:], m[:, 0:3, :], m[:, 1:4, :], OP.mult)
    cpA = st([CS, 3, B], FP16, "cpA")
    nc.vector.tensor_scalar_mul(cpA[:, 0, :], psA[:, 0, :], h[:, 0:1])
    nc.vector.tensor_scalar_mul(cpA[:, 1, :], psA[:, 1, :], h[:, 1:2])
    nc.scalar.mul(cpA[:, 2, :], psA[:, 2, :], h[:, 2:3])
    psB = st([CS, 2, B], FP16, "psB")
    nc.vector.tensor_tensor(psB[:], m[:, 0:2, :], m[:, 2:4, :], OP.mult)
    cpB = st([CS, 2, B], FP16, "cpB")
    nc.vector.tensor_scalar_mul(cpB[:, 0, :], psB[:, 0, :], h[:, 3:4])
    nc.vector.tensor_scalar_mul(cpB[:, 1, :], psB[:, 1, :], h[:, 4:5])
    # pair (0,3) on gpsimd
    psC = st([CS, B], FP16, "psC")
    nc.gpsimd.tensor_tensor(psC[:], m[:, 0, :], m[:, 3, :], OP.mult)
    cpC = st([CS, B], FP16, "cpC")
    nc.gpsimd.tensor_scalar_mul(cpC[:], psC[:], h[:, 5:6])

    # ---------------- num & den accumulation on PE (identity matmuls) ---
    Qsl = [Q01[:, 0, :], Q01[:, 1, :], Q23[:, 0, :], Q23[:, 1, :]]
    nmm = None
    for s in range(S):
        nmm = mm_chain(nmm, numb[:], eye[:], Qsl[s],
                       start=(s == 0), stop=(s == S - 1))
    denb = st([CS, B], F32, "denb", psum)
    dmm = None
    for s in range(S):
        dmm = mm_chain(dmm, denb[:], eye[:], Qsl[s],
                       start=(s == 0), stop=False)
    dmm = mm_chain(dmm, denb[:], eye[:], cpC[:], start=False, stop=False)
    for j in range(3):
        dmm = mm_chain(dmm, denb[:], eye[:], cpA[:, j, :], start=False, stop=False)
    for j in range(2):
        dmm = mm_chain(dmm, denb[:], eye[:], cpB[:, j, :], start=False, stop=(j == 1))

    # u = num * rnx (off critical path)
    rnx_sb = st([CS, B], FP16, "rnx_sb")
    nc.scalar.copy(rnx_sb[:], rnx_bc)
    u = st([CS, B], FP16, "u")
    nc.vector.tensor_tensor(u[:], numb[:], rnx_sb[:], OP.mult)

    srd = st([CS, B], FP16, "srd")
    nc.scalar.activation(srd[:], denb[:], AF.Abs_reciprocal_sqrt)
    nc.vector.tensor_tensor(ot[0:CS, 0, 0, :], u[:], srd[:], OP.mult)

    # output: SWDGE descriptors prepared after the ot write (the simulator
    # reads the source at prep position), fired by trigger_dma — skips the
    # HWDGE descriptor stage + DGE ramp on the critical tail.
    sem_ot = nc.alloc_semaphore("dma_ot")
    nc.gpsimd.kv_writeback(
        out_d[:, :, :, :], ot[:, :, :, :], ctx0[:],
        prepare_only=True, sem=sem_ot,
    )
    nc.gpsimd.trigger_dma(count=None)


def _build_nc():
    nc = bacc.Bacc(
        "TRN2",
        target_bir_lowering=False,
        debug=False,
        enable_asserts=False,
        num_devices=NCORES,
    )
    with tile.TileContext(nc) as tc:
        with (
            tc.tile_pool(name="main", bufs=1) as pool,
            tc.tile_pool(name="psum", bufs=1, space="PSUM") as psum,
        ):
            _emit_body(nc, pool, psum)
    nc.compile()
    _fix_swdge_waits(nc)
    return nc


def _fix_swdge_waits(nc):
    """Point consumer waits at the sems the SWDGE descriptors actually fire.

    Tile routes data deps on prepared-SWDGE outputs through per-lane DMASW<i>
    semaphores, but the hardware descriptor encodes exactly one sem — the
    user sem passed via ``sem=`` (on_update[0], +16 at transfer end). Tile
    never attaches the DMASW increment for gen_mode==1 preps, leaving those
    DMASW waits unsatisfiable. Rewrite each unsatisfied ``DMASW<i> >= 16``
    wait to the user sem of the prep on that lane (lanes assigned
    round-robin over Pool DMA instructions in program order).
    """
    import re

    fn = nc.m.functions[0]
    lane_sem = {}
    updated = set()
    n_dma = 0
    for blk in fn.blocks:
        for ins in blk.instructions:
            if ins.sync_info is None:
                continue
            for upd in ins.sync_info.on_update:
                if upd.ant_name:
                    updated.add(upd.ant_name)
            if ins.engine == mybir.EngineType.Pool and (
                    type(ins).__name__ in ("InstDMACopy", "InstDMAGatherAnt",
                                           "InstKVWritebackAnt",
                                           "InstDMAScatterAddAnt",
                                           "InstPagedWritebackAnt")):
                if getattr(ins, "gen_mode", 0) == 1:
                    lane_sem[n_dma] = ins.sync_info.on_update[0]
                n_dma += 1
    for blk in fn.blocks:
        for ins in blk.instructions:
            if ins.sync_info is None:
                continue
            for w in ins.sync_info.on_wait:
                m = re.match(r"DMASW(\d+)_", w.ant_name or "")
                if not m or w.ant_name in updated:
                    continue
                lane = int(m.group(1))
                assert w.wait_value == 16, (ins.name, w.ant_name, w.wait_value)
                assert lane in lane_sem, (ins.name, w.ant_name, lane_sem)
                u = lane_sem[lane]
                w.id = u.id
                w.ant_name = u.ant_name


def _get_nc():
    if "nc" not in _CACHED:
        _CACHED["nc"] = _build_nc()
    return _CACHED["nc"]


def _make_in_maps(x, W):
    x = np.ascontiguousarray(np.asarray(x, dtype=np.float32))
    W = np.ascontiguousarray(np.asarray(W, dtype=np.float32))
    xT = x.T.astype(np.float16)  # [D, B]
    in_maps = []
    for i in range(NCORES):
        Ws = W[i * CS:(i + 1) * CS].astype(np.float16)      # [CS, S, D]
        wT = Ws.transpose(2, 1, 0).reshape(D, R)            # [D, s*CS+c]
        xw = np.ascontiguousarray(np.concatenate([xT, wT], axis=1))
        in_maps.append({"xw": xw})
    return in_maps


def run(x, W, trace=False):
    nc = _get_nc()
    in_maps = _make_in_maps(x, W)
    res = run_bass_kernel_spmd(
        nc, in_maps, core_ids=list(range(NCORES)), trace=trace
    )
    shards = []
    for i in range(NCORES):
        o = np.asarray(res.results[i]["out"]).reshape(128, B)[0:CS, :]
        shards.append(o.astype(np.float32))
    out = np.concatenate([s.T for s in shards], axis=1)  # [B, C]
    return np.ascontiguousarray(out.astype(np.float32)), res


def kernel(x, W):
    out, _ = run(x, W, trace=False)
    return out
